# revision 28
# baseline (speedup 1.0000x reference)
"""Trainium2 Bass kernel for nn_CapsuleLinear (k-means 'dot' routing, 3 iters).

Math (per example b):
  priors[o,i,v] = sum_l W[o,i,v,l] * x[b,i,l]
  out0 = mean_i priors
  3x: n = normalize(out); logits[o,i] = sum_v priors*n; probs = softmax_o(logits);
      out[o,v] = sum_i probs*priors
  result = squash(out) + bias

Sharding: data-parallel over batch B=64 across 8 cores (8 examples/core).

Per-core layout (P = 128 partitions = (i_p in 0..15, b in 0..7), p = i_p*8+b):
  priors SBUF fp16 [128, ib=32, v=16, o=64], full i = ib*16 + i_p.

HW-measured engine facts that shaped this version:
  - DVE tensor_tensor on packed fp16 SBUF = 0.53 ns/elem (2x mode); fp32 /
    tensor_reduce / scalar_tensor_tensor = 1x.  All bulk elementwise work is
    fp16 TT on the DVE.
  - Pool tensor_tensor is ~4-6 ns/elem AND contends with the DVE for SBUF
    ports (DVE drops to ~0.8-1.1 ns/elem while Pool runs) — Pool gets no
    bulk work; it only folds half the out0 partials during the DMA-bound
    phase 1.
  - The PE drops to its 1.2 GHz mid-pstate unless continuously busy >3us, so
    dependency-free filler matmuls keep it ramped across the DVE-bound
    stretches; the i-reduction ones-matmuls then run at 2.4 GHz.
  - ACT function-table switches cost 1.3us, and no early-listed table set
    holds both Exp and Ln/Sqrt.  rsqrt/sqrt are computed on the DVE with the
    int32 bit trick + 2 Newton steps, so ACT only ever needs Copy/Exp (one
    table load for the whole kernel).
  - softmax: exp runs per-ib on ACT with the HW accumulator producing
    z = sum_o exp for free; probs = elog * (1/z) also runs per-ib on ACT as
    a Copy with a per-partition scale AP.  The DVE only does the reciprocal.
"""

import os

import numpy as np

import concourse.bacc as bacc
import concourse.tile as tile
from concourse import mybir
from concourse.bass_utils import run_bass_kernel_spmd

B, I, O, V, L = 64, 512, 64, 16, 8
NCORES = 8
BL = B // NCORES  # 8 examples per core
IB = I // 16  # 32 blocks of 16 i's
CHUNKS = [(0, 12), (12, 24), (24, 30), (30, 32)]  # small last chunks: PE tail

f32 = mybir.dt.float32
f16 = mybir.dt.float16
i32 = mybir.dt.int32

RSQRT_MAGIC = 0x5F3759DF

LAST_RESULT = None  # stash of BassKernelResults for test harness


def _build_kernel():
    nc = bacc.Bacc(
        "TRN2",
        target_bir_lowering=False,
        debug=False,
        enable_asserts=False,
        num_devices=NCORES,
    )
    w2_d = nc.dram_tensor("w2", [IB, 128, O * V], f16, kind="ExternalInput")
    xdg_d = nc.dram_tensor("xdg", [IB, 128, 128], f16, kind="ExternalInput")
    ones_d = nc.dram_tensor("onesd", [128, 128], f16, kind="ExternalInput")
    bias_d = nc.dram_tensor("biasT", [V, O], f32, kind="ExternalInput")
    out_d = nc.dram_tensor("out", [BL, V, O], f32, kind="ExternalOutput")

    with tile.TileContext(nc) as tc:
        _body(nc, tc, w2_d, xdg_d, ones_d, bias_d, out_d)
    nc.compile()
    return nc


def _body(nc, tc, w2_d, xdg_d, ones_d, bias_d, out_d):
    AL = mybir.AluOpType
    AF = mybir.ActivationFunctionType

    from contextlib import ExitStack

    with ExitStack() as ctx:
        big = ctx.enter_context(tc.tile_pool(name="big", bufs=1))
        wp = ctx.enter_context(tc.tile_pool(name="wp", bufs=3))
        xp = ctx.enter_context(tc.tile_pool(name="xp", bufs=2))
        sm = ctx.enter_context(tc.tile_pool(name="sm", bufs=1))
        pr_ps = ctx.enter_context(tc.tile_pool(name="prps", bufs=3, space="PSUM"))
        out_ps = ctx.enter_context(tc.tile_pool(name="outps", bufs=1, space="PSUM"))

        # ---- persistent tiles ----
        priors = big.tile([128, IB, V, O], f16)
        prod = big.tile([128, IB, V, O], f16)
        logits = big.tile([128, IB, O], f16)
        elog = big.tile([128, IB, O], f16)
        probs = big.tile([128, IB, O], f16)
        rzfull = big.tile([128, IB, O], f16)
        ones_t = big.tile([128, 128], f16)
        bias_t = big.tile([BL, V, O], f32)
        accd = big.tile([128, V, O], f16)  # out0 accumulator (DVE half)
        accp = big.tile([128, V, O], f16)  # out0 accumulator (Pool half)
        bm2 = big.tile([128, 1], f32)  # exp bias (-2, cancels in probs)
        magic = big.tile([128, O], i32)  # rsqrt bit-trick constant
        nc.vector.memset(bm2[:], -2.0)
        nc.vector.memset(magic[:], RSQRT_MAGIC)

        nc.sync.dma_start(out=ones_t[:], in_=ones_d[:])
        nc.sync.dma_start(
            out=bias_t[:], in_=bias_d[:].unsqueeze(0).broadcast_to([BL, V, O])
        )

        def rsqrt(y, x, tag):
            # y = x**-0.5 on the DVE only: int32 bit trick + 2 Newton steps
            # (rel err ~4e-6).  x fp32 [128, O], y fp32 [128, O].
            sh = sm.tile([128, O], i32, tag=tag + "_sh")
            nc.vector.tensor_scalar(
                out=sh[:], in0=x[:].bitcast(i32), scalar1=1, scalar2=None,
                op0=AL.arith_shift_right,
            )
            nc.vector.tensor_sub(y[:].bitcast(i32), magic[:], sh[:])
            t1 = sm.tile([128, O], f32, tag=tag + "_t1")
            for _ in range(2):
                nc.vector.tensor_mul(t1[:], y[:], y[:])
                nc.vector.tensor_mul(t1[:], t1[:], x[:])
                nc.vector.tensor_scalar(
                    out=t1[:], in0=t1[:], scalar1=-0.5, scalar2=1.5,
                    op0=AL.mult, op1=AL.add,
                )
                nc.vector.tensor_mul(y[:], y[:], t1[:])

        # ---- phase 1: priors + out0 ----
        out0 = out_ps.tile([128, V, O], f32, bufs=1)
        out0f = out0[:].rearrange("p v o -> p (v o)")

        # PSUM->SBUF copy engines per 8 ibs: 5x ACT, 3x DVE (Pool can't read PSUM)
        copy_eng = [0, 0, 1, 0, 1, 0, 0, 1]

        # out0 = sum_ib priors: fp16 2x TT adds, 20 ibs on the DVE and 12 on
        # Pool (the PE's 379ns/512-col matmuls are too expensive for this; it
        # only folds the two accumulators at the end)
        POOL_IBS = {ib for ib in range(IB) if ib % 8 in (3, 5, 7)}

        def out0_fold(ib):
            if ib in POOL_IBS:
                if ib == 3:
                    pass  # paired into the ib==5 seed
                elif ib == 5:
                    nc.gpsimd.tensor_add(accp[:], priors[:, 3], priors[:, 5])
                else:
                    nc.gpsimd.tensor_add(accp[:], accp[:], priors[:, ib])
            else:
                if ib == 0:
                    pass  # paired into the ib==1 seed
                elif ib == 1:
                    nc.vector.tensor_add(accd[:], priors[:, 0], priors[:, 1])
                else:
                    nc.vector.tensor_add(accd[:], accd[:], priors[:, ib])

        LAG = 3  # emit out0 folds LAG ibs late so the PE never waits on copies
        # DMA 4 ibs per dma_start: the SP sequencer costs ~600ns per trigger,
        # so per-ib triggers would pace phase 1 well below DMA bandwidth
        for g in range(IB // 4):
            w4 = wp.tile([128, 4, O * V], f16, tag="w")
            nc.sync.dma_start(out=w4[:], in_=w2_d[4 * g : 4 * g + 4].transpose([1, 0, 2]))
            xd4 = xp.tile([128, 4, 128], f16, tag="xd")
            nc.sync.dma_start(
                out=xd4[:], in_=xdg_d[4 * g : 4 * g + 4].transpose([1, 0, 2])
            )
            for j in range(4):
                ib = 4 * g + j
                pp = pr_ps.tile([128, O * V], f32, tag="pp")
                for h in range(2):
                    sl = slice(h * 512, (h + 1) * 512)
                    nc.tensor.matmul(
                        pp[:, sl], xd4[:, j], w4[:, j, sl], start=True, stop=True
                    )
                # PSUM (o,v) -> SBUF priors[:, ib] in (v, o) order, cast to fp16
                ppv = pp[:].rearrange("p (o v) -> p v o", o=O)
                if copy_eng[ib % 8] == 0:
                    nc.scalar.copy(out=priors[:, ib], in_=ppv)
                else:
                    nc.vector.tensor_copy(out=priors[:, ib], in_=ppv)
                if ib >= LAG:
                    out0_fold(ib - LAG)
        for ib in range(IB - LAG, IB):
            out0_fold(ib)

        # fold the fp16 partial sums into out0 (also re-broadcasts over i_p)
        for acc in (accd, accp):
            accf = acc[:].rearrange("p v o -> p (v o)")
            for h in range(2):
                sl = slice(h * 512, (h + 1) * 512)
                nc.tensor.matmul(
                    out0f[:, sl],
                    ones_t[:],
                    accf[:, sl],
                    start=(acc is accd),
                    stop=(acc is accp),
                    skip_group_check=True,
                )

        # ---- phase 2: routing iterations ----
        def emit_norm(out_vo):
            # ntile = out/||out|| from PSUM fp32 out_vo [128, V, O]
            outh = sm.tile([128, V, O], f16, tag="outh")
            # outh = out/4 so the fp16 sq partial sums stay in range
            nc.scalar.activation(out=outh[:], in_=out_vo, func=AF.Copy, scale=0.25)
            sqh = sm.tile([128, V, O], f16, tag="sqh")
            nc.vector.tensor_mul(sqh[:], outh[:], outh[:])
            nc.vector.tensor_add(sqh[:, 0:8], sqh[:, 0:8], sqh[:, 8:16])
            nc.vector.tensor_add(sqh[:, 0:4], sqh[:, 0:4], sqh[:, 4:8])
            nc.vector.tensor_add(sqh[:, 0:2], sqh[:, 0:2], sqh[:, 2:4])
            nsq = sm.tile([128, O], f32, tag="nsq")
            nc.vector.tensor_add(nsq[:], sqh[:, 0], sqh[:, 1])
            rn32 = sm.tile([128, O], f32, tag="rn32")
            rsqrt(rn32, nsq, "rs")
            rn = sm.tile([128, O], f16, tag="rn")
            nc.vector.tensor_copy(out=rn[:], in_=rn32[:])
            # ntile = outh * rn = (out/4) * (4/||out||) = out/||out||
            ntile = sm.tile([128, V, O], f16, tag="ntile")
            nc.vector.tensor_mul(
                ntile[:], outh[:], rn[:].unsqueeze(1).broadcast_to([128, V, O])
            )
            return ntile

        def emit_iter(out_vo):
            ntile = emit_norm(out_vo)
            out_new = pr_ps.tile([128, O * V], f32, tag="pp")
            z = sm.tile([128, IB], f32, tag="z")
            rz = sm.tile([128, IB], f32, tag="rz")

            def chunk_front(c):
                lo, hi = CHUNKS[c]
                s = slice(lo, hi)
                n = hi - lo
                # prod = priors * n (bcast over ib), then v-tree -> logits
                nc.vector.tensor_mul(
                    prod[:, s],
                    priors[:, s],
                    ntile[:].unsqueeze(1).broadcast_to([128, n, V, O]),
                )
                nc.vector.tensor_add(prod[:, s, 0:8], prod[:, s, 0:8], prod[:, s, 8:16])
                nc.vector.tensor_add(prod[:, s, 0:4], prod[:, s, 0:4], prod[:, s, 4:8])
                nc.vector.tensor_add(prod[:, s, 0:2], prod[:, s, 0:2], prod[:, s, 2:4])
                nc.vector.tensor_add(logits[:, s], prod[:, s, 0], prod[:, s, 1])
                # exp per ib on ACT; the HW accumulator yields z = sum_o exp
                for ib in range(lo, hi):
                    nc.scalar.activation(
                        out=elog[:, ib],
                        in_=logits[:, ib],
                        func=AF.Exp,
                        bias=bm2[:],
                        accum_out=z[:, ib : ib + 1],
                    )

            def chunk_back(c):
                lo, hi = CHUNKS[c]
                s = slice(lo, hi)
                n = hi - lo
                nc.vector.reciprocal(rz[:, s], z[:, s])
                # 1/z broadcast over o, materialized fp16 on the (otherwise
                # idle) Pool engine so the probs mul is a 2x TT on the DVE.
                # ACT would queue it behind the next chunk's exps.
                nc.gpsimd.tensor_copy(
                    out=rzfull[:, s],
                    in_=rz[:, s].unsqueeze(2).broadcast_to([128, n, O]),
                )
                nc.vector.tensor_mul(probs[:, s], elog[:, s], rzfull[:, s])
                # prod2 = priors * probs (bcast over v)
                nc.vector.tensor_mul(
                    prod[:, s],
                    priors[:, s],
                    probs[:, s].unsqueeze(2).broadcast_to([128, n, V, O]),
                )
                # i-reduction on the PE
                for ib in range(lo, hi):
                    pslc = prod[:, ib].rearrange("p v o -> p (v o)")
                    for h in range(2):
                        sl = slice(h * 512, (h + 1) * 512)
                        nc.tensor.matmul(
                            out_new[:, sl],
                            ones_t[:],
                            pslc[:, sl],
                            start=(ib == 0),
                            stop=(ib == IB - 1),
                            skip_group_check=True,
                        )

            chunk_front(0)
            chunk_front(1)
            chunk_back(0)
            chunk_back(1)
            chunk_front(2)
            chunk_front(3)
            chunk_back(2)
            chunk_back(3)
            return out_new

        out_prev_vo = out0[:]
        for t in range(3):
            out_new = emit_iter(out_prev_vo)
            out_prev_vo = out_new[:].rearrange("p (v o) -> p v o", v=V)

        # ---- squash + bias on partitions 0..7 (b rows) ----
        outh = sm.tile([128, V, O], f16, tag="outh")
        nc.scalar.copy(out=outh[:], in_=out_prev_vo)
        sqh = sm.tile([128, V, O], f16, tag="sqh")
        nc.vector.tensor_mul(sqh[:], outh[:], outh[:])
        nc.vector.tensor_add(sqh[:, 0:8], sqh[:, 0:8], sqh[:, 8:16])
        nc.vector.tensor_add(sqh[:, 0:4], sqh[:, 0:4], sqh[:, 4:8])
        nc.vector.tensor_add(sqh[:, 0:2], sqh[:, 0:2], sqh[:, 2:4])
        nsq = sm.tile([128, O], f32, tag="nsq")
        nc.vector.tensor_add(nsq[:], sqh[:, 0], sqh[:, 1])
        rq = sm.tile([128, O], f32, tag="rq")
        rsqrt(rq, nsq, "rst")
        norm = sm.tile([128, O], f32, tag="norm")
        nc.vector.tensor_mul(norm[:], nsq[:], rq[:])  # sqrt = x * rsqrt(x)
        den = sm.tile([128, O], f32, tag="den")
        nc.vector.tensor_scalar_add(den[:], nsq[:], 1.0)
        rden = sm.tile([128, O], f32, tag="rden")
        nc.vector.reciprocal(rden[:], den[:])
        scl = sm.tile([128, O], f32, tag="scl")
        nc.vector.tensor_mul(scl[:], norm[:], rden[:])

        outf = sm.tile([BL, V, O], f32, tag="outf")
        nc.vector.tensor_mul(
            outf[:],
            out_prev_vo[0:BL],
            scl[0:BL].unsqueeze(1).broadcast_to([BL, V, O]),
        )
        nc.vector.tensor_add(outf[:], outf[:], bias_t[:])
        nc.sync.dma_start(out=out_d[:], in_=outf[:])


_NC_CACHE = []


def _get_nc():
    if not _NC_CACHE:
        _NC_CACHE.append(_build_kernel())
    return _NC_CACHE[0]


def kernel(x, weight, bias):
    global LAST_RESULT
    x = np.asarray(x, dtype=np.float32)
    weight = np.asarray(weight, dtype=np.float32)
    bias = np.asarray(bias, dtype=np.float32)

    # W2[ib, (i_sub, l), (o, v)] = W[o, ib*16+i_sub, v, l]
    w2 = (
        np.ascontiguousarray(weight.transpose(1, 3, 0, 2))
        .reshape(IB, 128, O * V)
        .astype(np.float16)
    )
    biasT = np.ascontiguousarray(bias.T)  # [V, O]

    idx = np.arange(128)
    onesd = (idx[:, None] % BL == idx[None, :] % BL).astype(np.float16)

    in_maps = []
    for c in range(NCORES):
        xc = x[c * BL : (c + 1) * BL]  # [BL, I, L]
        xt = np.ascontiguousarray(xc.transpose(1, 2, 0))  # (i, l, b)
        xt4 = xt.reshape(IB, 16, L, BL)
        xdg = np.zeros((IB, 128, 128), dtype=np.float16)
        for s in range(16):
            xdg[:, s * L : (s + 1) * L, s * BL : (s + 1) * BL] = xt4[:, s].astype(
                np.float16
            )
        in_maps.append({"w2": w2, "xdg": xdg, "onesd": onesd, "biasT": biasT})

    nc = _get_nc()
    try:
        res = run_bass_kernel_spmd(nc, in_maps, core_ids=list(range(NCORES)))
    except ModuleNotFoundError:
        # BASS_TRACE was set but this environment lacks the axon NTFF hook
        # module; rerun without tracing.
        os.environ["BASS_NEVER_TRACE"] = "1"
        res = run_bass_kernel_spmd(nc, in_maps, core_ids=list(range(NCORES)))
    LAST_RESULT = res

    outs = []
    for r in res.results:
        o = r["out"]  # [BL, V, O]
        outs.append(np.ascontiguousarray(o.transpose(0, 2, 1)))  # [BL, O, V]
    return np.concatenate(outs, axis=0).astype(np.float32)


if __name__ == "__main__":
    rng = np.random.default_rng(0)
    x = rng.standard_normal((B, I, L), dtype=np.float32)
    w = rng.standard_normal((O, I, V, L), dtype=np.float32) * 0.1
    b = rng.standard_normal((O, V), dtype=np.float32) * 0.1
    out = kernel(x, w, b)
    print("out shape", out.shape, out.dtype)


# revision 31
# speedup vs baseline: 1.0520x; 1.0520x over previous
"""Trainium2 Bass kernel for nn_CapsuleLinear (k-means 'dot' routing, 3 iters).

Math (per example b):
  priors[o,i,v] = sum_l W[o,i,v,l] * x[b,i,l]
  out0 = mean_i priors
  3x: n = normalize(out); logits[o,i] = sum_v priors*n; probs = softmax_o(logits);
      out[o,v] = sum_i probs*priors
  result = squash(out) + bias

Sharding: data-parallel over batch B=64 across 8 cores (8 examples/core).

Per-core layout (P = 128 partitions = (i_p in 0..15, b in 0..7), p = i_p*8+b):
  priors SBUF fp16 [128, ib=32, v=16, o=64], full i = ib*16 + i_p.

HW-measured engine facts that shaped this version:
  - DVE tensor_tensor on packed fp16 SBUF = 0.53 ns/elem (2x mode); fp32 /
    tensor_reduce / scalar_tensor_tensor = 1x.  All bulk elementwise work is
    fp16 TT on the DVE.
  - Pool tensor_tensor is ~4-6 ns/elem AND contends with the DVE for SBUF
    ports (DVE drops to ~0.8-1.1 ns/elem while Pool runs) — Pool gets no
    bulk work; it only folds half the out0 partials during the DMA-bound
    phase 1.
  - The PE drops to its 1.2 GHz mid-pstate unless continuously busy >3us, so
    dependency-free filler matmuls keep it ramped across the DVE-bound
    stretches; the i-reduction ones-matmuls then run at 2.4 GHz.
  - ACT function-table switches cost 1.3us, and no early-listed table set
    holds both Exp and Ln/Sqrt.  rsqrt/sqrt are computed on the DVE with the
    int32 bit trick + 2 Newton steps, so ACT only ever needs Copy/Exp (one
    table load for the whole kernel).
  - softmax: exp runs per-ib on ACT with the HW accumulator producing
    z = sum_o exp for free; probs = elog * (1/z) also runs per-ib on ACT as
    a Copy with a per-partition scale AP.  The DVE only does the reciprocal.
"""

import os

import numpy as np

import concourse.bacc as bacc
import concourse.tile as tile
from concourse import mybir
from concourse.bass_utils import run_bass_kernel_spmd

B, I, O, V, L = 64, 512, 64, 16, 8
NCORES = 8
BL = B // NCORES  # 8 examples per core
IB = I // 16  # 32 blocks of 16 i's
CHUNKS = [(0, 12), (12, 24), (24, 30), (30, 32)]  # small last chunks: PE tail

f32 = mybir.dt.float32
f16 = mybir.dt.float16
i32 = mybir.dt.int32

RSQRT_MAGIC = 0x5F3759DF

LAST_RESULT = None  # stash of BassKernelResults for test harness


def _build_kernel():
    nc = bacc.Bacc(
        "TRN2",
        target_bir_lowering=False,
        debug=False,
        enable_asserts=False,
        num_devices=NCORES,
    )
    w2_d = nc.dram_tensor("w2", [IB, 128, O * V], f16, kind="ExternalInput")
    xdg_d = nc.dram_tensor("xdg", [IB, 128, 128], f16, kind="ExternalInput")
    ones_d = nc.dram_tensor("onesd", [128, 128], f16, kind="ExternalInput")
    bias_d = nc.dram_tensor("biasT", [V, O], f32, kind="ExternalInput")
    out_d = nc.dram_tensor("out", [BL, V, O], f32, kind="ExternalOutput")

    with tile.TileContext(nc) as tc:
        _body(nc, tc, w2_d, xdg_d, ones_d, bias_d, out_d)
    nc.compile()
    return nc


def _body(nc, tc, w2_d, xdg_d, ones_d, bias_d, out_d):
    AL = mybir.AluOpType
    AF = mybir.ActivationFunctionType

    from contextlib import ExitStack

    with ExitStack() as ctx:
        big = ctx.enter_context(tc.tile_pool(name="big", bufs=1))
        wp = ctx.enter_context(tc.tile_pool(name="wp", bufs=3))
        xp = ctx.enter_context(tc.tile_pool(name="xp", bufs=2))
        sm = ctx.enter_context(tc.tile_pool(name="sm", bufs=1))
        pr_ps = ctx.enter_context(tc.tile_pool(name="prps", bufs=3, space="PSUM"))
        out_ps = ctx.enter_context(tc.tile_pool(name="outps", bufs=1, space="PSUM"))

        # ---- persistent tiles ----
        priors = big.tile([128, IB, V, O], f16)
        prod = big.tile([128, IB, V, O], f16)
        logits = big.tile([128, IB, O], f16)
        elog = big.tile([128, IB, O], f16)
        probs = big.tile([128, IB, O], f16)
        rzfull = big.tile([128, IB, O], f16)
        ones_t = big.tile([128, 128], f16)
        bias_t = big.tile([BL, V, O], f32)
        accd = big.tile([128, V, O], f16)  # out0 accumulator (DVE half)
        accp = big.tile([128, V, O], f16)  # out0 accumulator (Pool half)
        bm2 = big.tile([128, 1], f32)  # exp bias (-2, cancels in probs)
        magic = big.tile([128, O], i32)  # rsqrt bit-trick constant
        nc.vector.memset(bm2[:], -2.0)
        nc.vector.memset(magic[:], RSQRT_MAGIC)

        nc.sync.dma_start(out=ones_t[:], in_=ones_d[:])
        nc.sync.dma_start(
            out=bias_t[:], in_=bias_d[:].unsqueeze(0).broadcast_to([BL, V, O])
        )

        def rsqrt(y, x, tag):
            # y = x**-0.5 on the DVE only: int32 bit trick + 2 Newton steps
            # (rel err ~4e-6).  x fp32 [128, O], y fp32 [128, O].
            sh = sm.tile([128, O], i32, tag=tag + "_sh")
            nc.vector.tensor_scalar(
                out=sh[:], in0=x[:].bitcast(i32), scalar1=1, scalar2=None,
                op0=AL.arith_shift_right,
            )
            nc.vector.tensor_sub(y[:].bitcast(i32), magic[:], sh[:])
            t1 = sm.tile([128, O], f32, tag=tag + "_t1")
            for _ in range(2):
                nc.vector.tensor_mul(t1[:], y[:], y[:])
                nc.vector.tensor_mul(t1[:], t1[:], x[:])
                nc.vector.tensor_scalar(
                    out=t1[:], in0=t1[:], scalar1=-0.5, scalar2=1.5,
                    op0=AL.mult, op1=AL.add,
                )
                nc.vector.tensor_mul(y[:], y[:], t1[:])

        # ---- phase 1: priors + out0 ----
        out0 = out_ps.tile([128, V, O], f32, bufs=1)
        out0f = out0[:].rearrange("p v o -> p (v o)")

        # PSUM->SBUF copy engines per 8 ibs: 5x ACT, 3x DVE (Pool can't read PSUM)
        copy_eng = [0, 0, 1, 0, 1, 0, 0, 1]

        # out0 = sum_ib priors: fp16 2x TT adds, 20 ibs on the DVE and 12 on
        # Pool (the PE's 379ns/512-col matmuls are too expensive for this; it
        # only folds the two accumulators at the end)
        POOL_IBS = {ib for ib in range(IB) if ib % 8 in (3, 5, 7)}

        def out0_fold(ib):
            if ib in POOL_IBS:
                if ib == 3:
                    pass  # paired into the ib==5 seed
                elif ib == 5:
                    nc.gpsimd.tensor_add(accp[:], priors[:, 3], priors[:, 5])
                else:
                    nc.gpsimd.tensor_add(accp[:], accp[:], priors[:, ib])
            else:
                if ib == 0:
                    pass  # paired into the ib==1 seed
                elif ib == 1:
                    nc.vector.tensor_add(accd[:], priors[:, 0], priors[:, 1])
                else:
                    nc.vector.tensor_add(accd[:], accd[:], priors[:, ib])

        LAG = 3  # emit out0 folds LAG ibs late so the PE never waits on copies
        # DMA 4 ibs per dma_start: the SP sequencer costs ~600ns per trigger,
        # so per-ib triggers would pace phase 1 well below DMA bandwidth
        for g in range(IB // 4):
            w4 = wp.tile([128, 4, O * V], f16, tag="w")
            nc.sync.dma_start(out=w4[:], in_=w2_d[4 * g : 4 * g + 4].transpose([1, 0, 2]))
            xd4 = xp.tile([128, 4, 128], f16, tag="xd")
            nc.sync.dma_start(
                out=xd4[:], in_=xdg_d[4 * g : 4 * g + 4].transpose([1, 0, 2])
            )
            for j in range(4):
                ib = 4 * g + j
                pp = pr_ps.tile([128, O * V], f32, tag="pp")
                for h in range(2):
                    sl = slice(h * 512, (h + 1) * 512)
                    nc.tensor.matmul(
                        pp[:, sl], xd4[:, j], w4[:, j, sl], start=True, stop=True
                    )
                # PSUM (o,v) -> SBUF priors[:, ib] in (v, o) order, cast to fp16
                ppv = pp[:].rearrange("p (o v) -> p v o", o=O)
                if copy_eng[ib % 8] == 0:
                    nc.scalar.copy(out=priors[:, ib], in_=ppv)
                else:
                    nc.vector.tensor_copy(out=priors[:, ib], in_=ppv)
                if ib >= LAG:
                    out0_fold(ib - LAG)
        for ib in range(IB - LAG, IB):
            out0_fold(ib)

        # fold the fp16 partial sums into out0 (also re-broadcasts over i_p)
        for acc in (accd, accp):
            accf = acc[:].rearrange("p v o -> p (v o)")
            for h in range(2):
                sl = slice(h * 512, (h + 1) * 512)
                nc.tensor.matmul(
                    out0f[:, sl],
                    ones_t[:],
                    accf[:, sl],
                    start=(acc is accd),
                    stop=(acc is accp),
                    skip_group_check=True,
                )

        # ---- phase 2: routing iterations ----
        def emit_norm(out_vo):
            # ntile = out/||out|| from PSUM fp32 out_vo [128, V, O].
            # sqh reads the PSUM directly so the DVE starts the moment the
            # last i-reduce matmul lands; the ACT outh cast runs concurrently
            # (it is only needed at the very end, for ntile).
            sqh = sm.tile([128, V, O], f16, tag="sqh")
            nc.scalar.square(sqh[:], out_vo)
            outh = sm.tile([128, V, O], f16, tag="outh")
            nc.scalar.copy(out=outh[:], in_=out_vo)
            nc.vector.tensor_add(sqh[:, 0:8], sqh[:, 0:8], sqh[:, 8:16])
            nc.vector.tensor_add(sqh[:, 0:4], sqh[:, 0:4], sqh[:, 4:8])
            nc.vector.tensor_add(sqh[:, 0:2], sqh[:, 0:2], sqh[:, 2:4])
            nsq = sm.tile([128, O], f32, tag="nsq")
            nc.vector.tensor_add(nsq[:], sqh[:, 0], sqh[:, 1])
            rn32 = sm.tile([128, O], f32, tag="rn32")
            rsqrt(rn32, nsq, "rs")
            rn = sm.tile([128, O], f16, tag="rn")
            nc.vector.tensor_copy(out=rn[:], in_=rn32[:])
            ntile = sm.tile([128, V, O], f16, tag="ntile")
            nc.vector.tensor_mul(
                ntile[:], outh[:], rn[:].unsqueeze(1).broadcast_to([128, V, O])
            )
            return ntile

        def emit_iter(out_vo):
            ntile = emit_norm(out_vo)
            out_new = pr_ps.tile([128, O * V], f32, tag="pp")
            z = sm.tile([128, IB], f32, tag="z")
            rz = sm.tile([128, IB], f32, tag="rz")

            def chunk_front(c):
                lo, hi = CHUNKS[c]
                s = slice(lo, hi)
                n = hi - lo
                # prod = priors * n (bcast over ib), then v-tree -> logits
                nc.vector.tensor_mul(
                    prod[:, s],
                    priors[:, s],
                    ntile[:].unsqueeze(1).broadcast_to([128, n, V, O]),
                )
                nc.vector.tensor_add(prod[:, s, 0:8], prod[:, s, 0:8], prod[:, s, 8:16])
                nc.vector.tensor_add(prod[:, s, 0:4], prod[:, s, 0:4], prod[:, s, 4:8])
                nc.vector.tensor_add(prod[:, s, 0:2], prod[:, s, 0:2], prod[:, s, 2:4])
                nc.vector.tensor_add(logits[:, s], prod[:, s, 0], prod[:, s, 1])
                # exp per ib on ACT; the HW accumulator yields z = sum_o exp
                for ib in range(lo, hi):
                    nc.scalar.activation(
                        out=elog[:, ib],
                        in_=logits[:, ib],
                        func=AF.Exp,
                        bias=bm2[:],
                        accum_out=z[:, ib : ib + 1],
                    )

            def chunk_back(c):
                lo, hi = CHUNKS[c]
                s = slice(lo, hi)
                n = hi - lo
                nc.vector.reciprocal(rz[:, s], z[:, s])
                # 1/z broadcast over o, materialized fp16 on ACT so the
                # probs mul is a 2x TT on the DVE (Pool would contend with
                # the DVE for SBUF ports and slow everything down)
                nc.scalar.copy(
                    out=rzfull[:, s],
                    in_=rz[:, s].unsqueeze(2).broadcast_to([128, n, O]),
                )
                nc.vector.tensor_mul(probs[:, s], elog[:, s], rzfull[:, s])
                # prod2 = priors * probs (bcast over v)
                nc.vector.tensor_mul(
                    prod[:, s],
                    priors[:, s],
                    probs[:, s].unsqueeze(2).broadcast_to([128, n, V, O]),
                )
                # i-reduction on the PE
                for ib in range(lo, hi):
                    pslc = prod[:, ib].rearrange("p v o -> p (v o)")
                    for h in range(2):
                        sl = slice(h * 512, (h + 1) * 512)
                        nc.tensor.matmul(
                            out_new[:, sl],
                            ones_t[:],
                            pslc[:, sl],
                            start=(ib == 0),
                            stop=(ib == IB - 1),
                            skip_group_check=True,
                        )

            chunk_front(0)
            chunk_front(1)
            chunk_back(0)
            chunk_back(1)
            chunk_front(2)
            chunk_front(3)
            chunk_back(2)
            chunk_back(3)
            return out_new

        out_prev_vo = out0[:]
        for t in range(3):
            out_new = emit_iter(out_prev_vo)
            out_prev_vo = out_new[:].rearrange("p (v o) -> p v o", v=V)

        # ---- squash + bias on partitions 0..7 (b rows) ----
        outh = sm.tile([128, V, O], f16, tag="outh")
        nc.scalar.copy(out=outh[:], in_=out_prev_vo)
        sqh = sm.tile([128, V, O], f16, tag="sqh")
        nc.vector.tensor_mul(sqh[:], outh[:], outh[:])
        nc.vector.tensor_add(sqh[:, 0:8], sqh[:, 0:8], sqh[:, 8:16])
        nc.vector.tensor_add(sqh[:, 0:4], sqh[:, 0:4], sqh[:, 4:8])
        nc.vector.tensor_add(sqh[:, 0:2], sqh[:, 0:2], sqh[:, 2:4])
        nsq = sm.tile([128, O], f32, tag="nsq")
        nc.vector.tensor_add(nsq[:], sqh[:, 0], sqh[:, 1])
        rq = sm.tile([128, O], f32, tag="rq")
        rsqrt(rq, nsq, "rst")
        norm = sm.tile([128, O], f32, tag="norm")
        nc.vector.tensor_mul(norm[:], nsq[:], rq[:])  # sqrt = x * rsqrt(x)
        den = sm.tile([128, O], f32, tag="den")
        nc.vector.tensor_scalar_add(den[:], nsq[:], 1.0)
        rden = sm.tile([128, O], f32, tag="rden")
        nc.vector.reciprocal(rden[:], den[:])
        scl = sm.tile([128, O], f32, tag="scl")
        nc.vector.tensor_mul(scl[:], norm[:], rden[:])

        outf = sm.tile([BL, V, O], f32, tag="outf")
        nc.vector.tensor_mul(
            outf[:],
            out_prev_vo[0:BL],
            scl[0:BL].unsqueeze(1).broadcast_to([BL, V, O]),
        )
        nc.vector.tensor_add(outf[:], outf[:], bias_t[:])
        nc.sync.dma_start(out=out_d[:], in_=outf[:])


_NC_CACHE = []


def _get_nc():
    if not _NC_CACHE:
        _NC_CACHE.append(_build_kernel())
    return _NC_CACHE[0]


def kernel(x, weight, bias):
    global LAST_RESULT
    x = np.asarray(x, dtype=np.float32)
    weight = np.asarray(weight, dtype=np.float32)
    bias = np.asarray(bias, dtype=np.float32)

    # W2[ib, (i_sub, l), (o, v)] = W[o, ib*16+i_sub, v, l]
    w2 = (
        np.ascontiguousarray(weight.transpose(1, 3, 0, 2))
        .reshape(IB, 128, O * V)
        .astype(np.float16)
    )
    biasT = np.ascontiguousarray(bias.T)  # [V, O]

    idx = np.arange(128)
    onesd = (idx[:, None] % BL == idx[None, :] % BL).astype(np.float16)

    in_maps = []
    for c in range(NCORES):
        xc = x[c * BL : (c + 1) * BL]  # [BL, I, L]
        xt = np.ascontiguousarray(xc.transpose(1, 2, 0))  # (i, l, b)
        xt4 = xt.reshape(IB, 16, L, BL)
        xdg = np.zeros((IB, 128, 128), dtype=np.float16)
        for s in range(16):
            xdg[:, s * L : (s + 1) * L, s * BL : (s + 1) * BL] = xt4[:, s].astype(
                np.float16
            )
        in_maps.append({"w2": w2, "xdg": xdg, "onesd": onesd, "biasT": biasT})

    nc = _get_nc()
    try:
        res = run_bass_kernel_spmd(nc, in_maps, core_ids=list(range(NCORES)))
    except ModuleNotFoundError:
        # BASS_TRACE was set but this environment lacks the axon NTFF hook
        # module; rerun without tracing.
        os.environ["BASS_NEVER_TRACE"] = "1"
        res = run_bass_kernel_spmd(nc, in_maps, core_ids=list(range(NCORES)))
    LAST_RESULT = res

    outs = []
    for r in res.results:
        o = r["out"]  # [BL, V, O]
        outs.append(np.ascontiguousarray(o.transpose(0, 2, 1)))  # [BL, O, V]
    return np.concatenate(outs, axis=0).astype(np.float32)


if __name__ == "__main__":
    rng = np.random.default_rng(0)
    x = rng.standard_normal((B, I, L), dtype=np.float32)
    w = rng.standard_normal((O, I, V, L), dtype=np.float32) * 0.1
    b = rng.standard_normal((O, V), dtype=np.float32) * 0.1
    out = kernel(x, w, b)
    print("out shape", out.shape, out.dtype)
